# revision 13
# baseline (speedup 1.0000x reference)
"""Trainium2 Bass kernel for nn_LongDistanceAttention (GNN message passing).

Strategy (8 NeuronCores, SPMD, node/row sharding):
  Each core owns a 512-row block of nodes. All N x N score/attention work is
  done on the transposed layout [j(source, partitions), i(local rows, free)]:
    - stage-1 GAT: e.T[j,i] = lrelu(s_i[i] + s_j[j]); E = exp(e.T) * A.T-block;
      (E' @ [Wh | 1]).T accumulated on PE gives numerator and row-sum at once;
      h_local = gelu(U / Z). Softmax without max-subtraction (scores are small,
      validated |e|<6, |scores|<15).
    - h is all-gathered (both natural [N,257] with ones column, bf16, and
      transposed [256,N] f32 layouts).
    - k-hop masks: A^k reachability via dense matmuls in fp8 (exact: inputs are
      0/1, products/accumulation exact in fp32 PSUM), binarized after each hop.
      Transposed recurrence: M_{k} = A.T @ M_{k-1} with lhsT = A8 (full fp8 A,
      all-gathered, streamed from DRAM), rhs = previous binary mask column-block.
    - per hop: E_k = exp(scores.T) * mask_k (bf16), U.T = (h_aug.T)(E_k) on PE
      with ones column giving Z; out.T += U.T * (1/Z broadcast).
  Final: Y.T = W_out.T @ out.T + b_out, output per core [128, 512] = block.T.
"""

import os
import sys

import numpy as np

sys.path.insert(0, "/opt/trn_rl_repo")

import concourse.bass as bass  # noqa: E402
import concourse.mybir as mybir  # noqa: E402
import concourse.tile as tile  # noqa: E402
from concourse import bacc  # noqa: E402
from concourse.bass_utils import run_bass_kernel_spmd  # noqa: E402
from concourse.masks import make_identity  # noqa: E402

P = 128
N = 4096
NB = N // P            # 32 j-chunks
HID = 256
OUT_DIM = 128
NCORES = 8
LOC = N // NCORES      # 512 local rows per core
LB = LOC // P          # 4 local partition chunks
ALPHA = 0.2

F32 = mybir.dt.float32
F32R = mybir.dt.float32r
BF16 = mybir.dt.bfloat16
FP8 = mybir.dt.float8e4

# mask-matmul mode: "fp8dr" (DoubleRow, 2x) or "fp8" (plain)
MASK_MODE = os.environ.get("MASK_MODE", "fp8dr")

_CACHE = {}
last_in_maps = None




def build_kernel():
    nc = bacc.Bacc(
        "TRN2",
        target_bir_lowering=False,
        debug=False,
        enable_asserts=False,
        num_devices=NCORES,
    )

    # ---- kernel I/O ----
    X_d = nc.dram_tensor("X", [N, HID], F32, kind="ExternalInput")
    Xloc_d = nc.dram_tensor("X_loc", [LOC, HID], F32, kind="ExternalInput")
    Ablk_d = nc.dram_tensor("A_blk", [LOC, N], F32, kind="ExternalInput")
    Ws_d = nc.dram_tensor("W_s", [HID, HID], F32, kind="ExternalInput")
    r_d = nc.dram_tensor("r", [2 * HID, 1], F32, kind="ExternalInput")
    Wl_d = nc.dram_tensor("W_l", [HID, HID], F32, kind="ExternalInput")
    Wo_d = nc.dram_tensor("W_out", [HID, OUT_DIM], F32, kind="ExternalInput")
    bo_d = nc.dram_tensor("b_out", [OUT_DIM], F32, kind="ExternalInput")
    out_d = nc.dram_tensor("out", [OUT_DIM, LOC], F32, kind="ExternalOutput")

    # ---- internal DRAM ----
    a8_loc = nc.dram_tensor("a8_loc", [LOC, N], FP8)
    a8_all = nc.dram_tensor("a8_all", [N, N], FP8, addr_space="Shared")
    abf_loc = nc.dram_tensor("abf_loc", [LOC, N], BF16)
    haug_loc = nc.dram_tensor("haug_loc", [LOC, HID + 2], BF16)
    haug_all = nc.dram_tensor("haug_all", [N, HID + 2], BF16, addr_space="Shared")
    htl_loc = nc.dram_tensor("htl_loc", [HID, LOC], F32R)
    ht_all = nc.dram_tensor("ht_all", [HID * NCORES, LOC], F32R, addr_space="Shared")

    groups = [list(range(NCORES))]

    with tile.TileContext(nc) as tc:
        with (
            tc.tile_pool(name="const", bufs=1) as cpool,
            tc.tile_pool(name="small", bufs=1) as sm,
            tc.tile_pool(name="maskp", bufs=1) as mp,
            tc.tile_pool(name="wk", bufs=1) as wk,
            tc.tile_pool(name="pp", bufs=1, space="PSUM") as pp,
        ):
            # =========== constants / weights ===========
            ident = cpool.tile([P, P], F32)
            make_identity(nc, ident)
            ident_r = cpool.tile([P, P], F32R)
            nc.vector.tensor_copy(ident_r[:], ident[:])
            Ws_sb = cpool.tile([P, 2, HID], F32R)
            nc.sync.dma_start(Ws_sb[:], Ws_d.ap().rearrange("(k p) m -> p k m", p=P).bitcast(F32R))
            Wl_sb = cpool.tile([P, 2, HID], F32R)
            nc.sync.dma_start(Wl_sb[:], Wl_d.ap().rearrange("(k p) m -> p k m", p=P).bitcast(F32R))
            Wo_sb = cpool.tile([P, 2, OUT_DIM], F32R)
            nc.sync.dma_start(Wo_sb[:], Wo_d.ap().rearrange("(k p) m -> p k m", p=P).bitcast(F32R))
            r_sb = cpool.tile([P, 4], F32R)
            nc.sync.dma_start(r_sb[:], r_d.ap().rearrange("(c p) o -> p (c o)", p=P).bitcast(F32R))
            bo_sb = cpool.tile([P, 1], F32)
            nc.sync.dma_start(bo_sb[:], bo_d.ap().rearrange("(o p) -> p o", p=P))

            # mask tiles (persist across hops)
            M0 = mp.tile([P, NB, LOC], FP8, name="M0")          # A.T block
            M1 = mp.tile([P, NB, LOC], FP8, name="M1")          # (A^2).T block
            M2 = mp.tile([P, NB, LOC], FP8, name="M2")          # (A^3).T block

            # small persistent tiles
            hT = sm.tile([P, 2, LOC], F32R, name="hT")           # h_local.T
            hnat = sm.tile([P, LB, HID + 2], BF16, name="hnat")
            outT = sm.tile([P, 2, LOC], F32R, name="outT")

            # =========== phase 1: A-block prep + A8 all-gather ===========
            NH = N // 2
            with tc.tile_pool(name="atp", bufs=1) as atp:
                At_bf = atp.tile([P, NB, LOC], BF16, name="At_bf")
                with tc.tile_pool(name="aprep", bufs=1) as aprep:
                    for ic in range(LB):
                        for nh in range(2):
                            ablk = aprep.tile(
                                [P, NH], F32, tag="ablk", bufs=2, name="ablk"
                            )
                            nc.sync.dma_start(
                                ablk[:],
                                Ablk_d.ap()[
                                    ic * P : (ic + 1) * P, nh * NH : (nh + 1) * NH
                                ],
                            )
                            a8q = aprep.tile(
                                [P, NH], FP8, tag="a8q", bufs=2, name="a8q"
                            )
                            abfq = aprep.tile(
                                [P, NH], BF16, tag="abfq", bufs=2, name="abfq"
                            )
                            nc.vector.tensor_copy(a8q[:], ablk[:])
                            nc.vector.tensor_copy(abfq[:], ablk[:])
                            nc.sync.dma_start(
                                a8_loc.ap()[
                                    ic * P : (ic + 1) * P, nh * NH : (nh + 1) * NH
                                ],
                                a8q[:],
                            )
                            nc.sync.dma_start(
                                abf_loc.ap()[
                                    ic * P : (ic + 1) * P, nh * NH : (nh + 1) * NH
                                ],
                                abfq[:],
                            )
                nc.gpsimd.collective_compute(
                    "AllGather",
                    mybir.AluOpType.bypass,
                    ins=[a8_loc[:, :]],
                    outs=[a8_all[:, :]],
                    replica_groups=groups,
                )
                # transposed A block (bf16, via DMA transpose), then fp8 copy
                for jc in range(NB):
                    nc.sync.dma_start_transpose(
                        At_bf[:, jc, :], abf_loc[:, jc * P : (jc + 1) * P]
                    )
                for jc in range(NB):
                    nc.vector.tensor_copy(M0[:, jc], At_bf[:, jc])

                # =========== phase 2: Wh, s vectors ===========
                with tc.tile_pool(name="s1pool", bufs=1) as s1pool:
                    Wh_aug = s1pool.tile([P, NB, HID + 2], F32R)
                    onez = s1pool.tile([P, NB, 2], F32)
                    nc.vector.memset(onez[:, :, 0:1], 1.0)
                    nc.vector.memset(onez[:, :, 1:2], 0.0)
                    nc.vector.tensor_copy(Wh_aug[:, :, HID : HID + 2], onez[:])
                    s_nat = s1pool.tile([P, NB], F32)

                    for o in range(NB):
                        xchunk = wk.tile([P, HID], F32, tag="w512f", bufs=8)
                        nc.sync.dma_start(
                            xchunk[:], X_d.ap()[o * P : (o + 1) * P, :]
                        )
                        xt = wk.tile([P, 2, P], F32R, tag="w512f", bufs=8)
                        for k in range(2):
                            pt = pp.tile([P, P], F32, tag="mask", bufs=2, name="pt")
                            nc.tensor.transpose(
                                pt[:], xchunk[:, k * P : (k + 1) * P], ident[:]
                            )
                            nc.vector.tensor_copy(xt[:, k], pt[:])
                        # Wh.T columns for this node chunk (transient)
                        whc = wk.tile([P, 2, P], F32R, tag="w512f", bufs=8)
                        for m2 in range(2):
                            pw = pp.tile([P, P], F32, tag="st", bufs=2, name="pw")
                            for k in range(2):
                                nc.tensor.matmul(
                                    pw[:],
                                    Ws_sb[:, k, m2 * P : (m2 + 1) * P],
                                    xt[:, k],
                                    start=(k == 0),
                                    stop=(k == 1),
                                )
                            nc.vector.tensor_copy(whc[:, m2], pw[:])
                        # Wh natural rows for this node chunk
                        pa = pp.tile([P, HID], F32, tag="agg", bufs=2, name="pa")
                        for k in range(2):
                            nc.tensor.matmul(
                                pa[:],
                                xt[:, k],
                                Ws_sb[:, k, :],
                                start=(k == 0),
                                stop=(k == 1),
                            )
                        nc.vector.tensor_copy(Wh_aug[:, o, :HID], pa[:])
                        # s_j for this chunk: Wh[chunk] @ r2
                        psn = pp.tile([P, 2], F32, tag="bcast", bufs=1, name="psn")
                        rp = r_sb.rearrange("p (h c) -> p c h", c=2)
                        for k in range(2):
                            nc.tensor.matmul(
                                psn[:],
                                whc[:, k],
                                rp[:, k, :],
                                start=(k == 0),
                                stop=(k == 1),
                            )
                        nc.vector.tensor_copy(s_nat[:, o : o + 1], psn[:, 1:2])

                    # local Wh.T (from X_loc) for the s_i row
                    WhlT = s1pool.tile([P, 2, LOC], F32R)
                    for ic in range(LB):
                        xlc = wk.tile([P, HID], F32, tag="w512f", bufs=8)
                        nc.sync.dma_start(
                            xlc[:], Xloc_d.ap()[ic * P : (ic + 1) * P, :]
                        )
                        xlt = wk.tile([P, 2, P], F32R, tag="w512f", bufs=8)
                        for k in range(2):
                            pt2 = pp.tile([P, P], F32, tag="mask", bufs=2, name="pt2")
                            nc.tensor.transpose(
                                pt2[:], xlc[:, k * P : (k + 1) * P], ident[:]
                            )
                            nc.vector.tensor_copy(xlt[:, k], pt2[:])
                        for m2 in range(2):
                            pw2 = pp.tile([P, P], F32, tag="st", bufs=2, name="pw2")
                            for k in range(2):
                                nc.tensor.matmul(
                                    pw2[:],
                                    Ws_sb[:, k, m2 * P : (m2 + 1) * P],
                                    xlt[:, k],
                                    start=(k == 0),
                                    stop=(k == 1),
                                )
                            nc.vector.tensor_copy(
                                WhlT[:, m2, ic * P : (ic + 1) * P], pw2[:]
                            )

                    # s_i row [1, 512] for the local block
                    psr = pp.tile([2, LOC], F32, tag="aggz", bufs=1, name="psr")
                    rp2 = r_sb.rearrange("p (h c) -> p c h", c=2)
                    for k in range(2):
                        nc.tensor.matmul(
                            psr[:],
                            rp2[:, k, :],
                            WhlT[:, k, :],
                            start=(k == 0),
                            stop=(k == 1),
                        )
                    sir = s1pool.tile([1, LOC], F32)
                    nc.vector.tensor_copy(sir[:], psr[0:1, :])
                    B_sb = s1pool.tile([P, LOC], F32)
                    nc.gpsimd.partition_broadcast(B_sb[:], sir[:])

                    # =========== phase 3: stage-1 attention ===========
                    u0 = pp.tile([P, LOC], F32, tag="agg", bufs=2, name="u0")
                    u1 = pp.tile([P, LOC], F32, tag="agg", bufs=2, name="u1")
                    uz = pp.tile([2, LOC], F32, tag="aggz", bufs=1, name="uz")
                    for jc in range(NB):
                        # leaky_relu(s_i + s_j) = max(t, ALPHA*t), built on DVE
                        # (ACT Lrelu ignores the alpha parameter on this stack)
                        t1 = wk.tile([P, LOC], F32, tag="w512f", bufs=8)
                        nc.vector.tensor_scalar(
                            t1[:], B_sb[:], s_nat[:, jc : jc + 1], None,
                            mybir.AluOpType.add,
                        )
                        t2 = wk.tile([P, LOC], F32, tag="w512f", bufs=8)
                        nc.vector.tensor_scalar(
                            t2[:], B_sb[:], s_nat[:, jc : jc + 1], ALPHA,
                            mybir.AluOpType.add, mybir.AluOpType.mult,
                        )
                        ex = wk.tile([P, LOC], F32, tag="w512f", bufs=8)
                        nc.vector.tensor_max(out=ex[:], in0=t1[:], in1=t2[:])
                        ee = wk.tile([P, LOC], F32, tag="w512f", bufs=8)
                        nc.scalar.activation(
                            ee[:], ex[:], mybir.ActivationFunctionType.Exp
                        )
                        at32 = wk.tile([P, LOC], F32, tag="w512f", bufs=8)
                        nc.vector.tensor_copy(at32[:], At_bf[:, jc])
                        em = wk.tile([P, LOC], F32R, tag="w512f", bufs=8)
                        nc.vector.tensor_mul(out=em[:], in0=ee[:], in1=at32[:])
                        last = jc == NB - 1
                        nc.tensor.matmul(
                            u0[:], Wh_aug[:, jc, 0:P], em[:],
                            start=(jc == 0), stop=last,
                        )
                        nc.tensor.matmul(
                            u1[:], Wh_aug[:, jc, P : 2 * P], em[:],
                            start=(jc == 0), stop=last,
                        )
                        nc.tensor.matmul(
                            uz[:], Wh_aug[:, jc, HID : HID + 2], em[:],
                            start=(jc == 0), stop=last,
                        )

                    # normalize + gelu -> h_local.T [256, 512]
                    zr = s1pool.tile([1, LOC], F32)
                    nc.vector.reciprocal(zr[:], uz[0:1, :])
                    zb = s1pool.tile([P, LOC], F32)
                    nc.gpsimd.partition_broadcast(zb[:], zr[:])
                    for mt, um in enumerate((u0, u1)):
                        tnorm = wk.tile([P, LOC], F32, tag="w512f", bufs=8)
                        nc.vector.tensor_mul(out=tnorm[:], in0=um[:], in1=zb[:])
                        nc.scalar.activation(
                            hT[:, mt], tnorm[:], mybir.ActivationFunctionType.Gelu
                        )

            # ====== mask matmuls: M1 = bin(A.T @ M0), M2 = bin(A.T @ M1) ======
            # (emitted early: they only depend on the A8 all-gather + M0, so
            #  the PE can chew on them while h is gathered)
            def mask_matmul(rhs_tile, out_tile):
                a8_r = a8_all.ap()
                for mg in range(16):
                    pms = [
                        pp.tile([P, LOC], F32, tag="mask", bufs=2, name=f"pm{mi}")
                        for mi in range(2)
                    ]
                    if MASK_MODE == "fp8dr":
                        for kp in range(16):
                            a8t = wk.tile([P, 2, 2 * P], FP8, tag="a8t", bufs=4)
                            src = a8_r.rearrange(
                                "(kp ko p) n -> p ko kp n", p=P, ko=2
                            )
                            nc.sync.dma_start(
                                a8t[:],
                                src[:, :, kp, 2 * P * mg : 2 * P * (mg + 1)],
                            )
                            for mi in range(2):
                                nc.tensor.matmul(
                                    pms[mi][:],
                                    a8t[:, :, mi * P : (mi + 1) * P],
                                    rhs_tile[:, 2 * kp : 2 * kp + 2, :],
                                    start=(kp == 0),
                                    stop=(kp == 15),
                                    perf_mode=mybir.MatmulPerfMode.DoubleRow,
                                )
                    else:
                        for kc in range(NB):
                            a8t2 = wk.tile([P, 2 * P], FP8, tag="a8t", bufs=4)
                            src = a8_r.rearrange("(kc p) n -> p kc n", p=P)
                            nc.sync.dma_start(
                                a8t2[:],
                                src[:, kc, 2 * P * mg : 2 * P * (mg + 1)],
                            )
                            for mi in range(2):
                                nc.tensor.matmul(
                                    pms[mi][:],
                                    a8t2[:, mi * P : (mi + 1) * P],
                                    rhs_tile[:, kc, :],
                                    start=(kc == 0),
                                    stop=(kc == NB - 1),
                                )
                    for mi in range(2):
                        nc.vector.tensor_scalar(
                            out_tile[:, 2 * mg + mi],
                            pms[mi][:],
                            0.5,
                            None,
                            mybir.AluOpType.is_gt,
                        )

            mask_matmul(M0, M1)
            mask_matmul(M1, M2)

            # =========== phase 4: h all-gathers ===========
            nc.vector.memset(hnat[:, :, HID : HID + 1], 1.0)
            nc.vector.memset(hnat[:, :, HID + 1 : HID + 2], 0.0)
            for ic in range(LB):
                for fc in range(2):
                    pht = pp.tile([P, P], F32R, tag="st", bufs=2, name="pht")
                    nc.tensor.transpose(
                        pht[:], hT[:, fc, ic * P : (ic + 1) * P], ident_r[:]
                    )
                    nc.vector.tensor_copy(hnat[:, ic, fc * P : (fc + 1) * P], pht[:])
            nc.sync.dma_start(
                haug_loc.ap().rearrange("(c p) f -> p c f", p=P), hnat[:]
            )
            nc.gpsimd.collective_compute(
                "AllGather",
                mybir.AluOpType.bypass,
                ins=[haug_loc[:, :]],
                outs=[haug_all[:, :]],
                replica_groups=groups,
            )
            nc.sync.dma_start(htl_loc.ap().rearrange("(c p) n -> p c n", p=P), hT[:])
            nc.gpsimd.collective_compute(
                "AllGather",
                mybir.AluOpType.bypass,
                ins=[htl_loc[:, :]],
                outs=[ht_all[:, :]],
                replica_groups=groups,
            )

            with tc.tile_pool(name="hpool", bufs=1) as hp:
                h_aug = hp.tile([P, NB, HID + 2], BF16, name="h_aug")
                nc.sync.dma_start(
                    h_aug[:], haug_all.ap().rearrange("(o p) f -> p o f", p=P)
                )
                expS = hp.tile([P, NB, LOC], BF16, name="expS")

                # =========== phase 5: scores (loop-invariant) ===========
                with tc.tile_pool(name="scpool", bufs=1) as scpool:
                    WaT = scpool.tile([P, 2, N], F32R)
                    ht_r = ht_all.ap().rearrange("(o p) n -> p o n", p=P)
                    for c in range(NCORES):
                        hpair = wk.tile([P, 2, LOC], F32R, tag="hpair", bufs=2)
                        nc.sync.dma_start(hpair[:], ht_r[:, 2 * c : 2 * c + 2, :])
                        for m2 in range(2):
                            pwa = pp.tile([P, LOC], F32, tag="st", bufs=2, name="pwa")
                            for f in range(2):
                                nc.tensor.matmul(
                                    pwa[:],
                                    Wl_sb[:, f, m2 * P : (m2 + 1) * P],
                                    hpair[:, f, :],
                                    start=(f == 0),
                                    stop=(f == 1),
                                )
                            nc.vector.tensor_copy(
                                WaT[:, m2, c * LOC : (c + 1) * LOC], pwa[:]
                            )
                    for m in range(NB):
                        pst = pp.tile([P, LOC], F32, tag="st", bufs=2, name="pst")
                        for k in range(2):
                            nc.tensor.matmul(
                                pst[:],
                                WaT[:, k, m * P : (m + 1) * P],
                                hT[:, k, :],
                                start=(k == 0),
                                stop=(k == 1),
                            )
                        nc.scalar.activation(
                            expS[:, m], pst[:], mybir.ActivationFunctionType.Exp
                        )

                # =========== phase 6: k-hop aggregation ===========
                def hop(e_getter, first):
                    u0h = pp.tile([P, LOC], F32, tag="agg", bufs=2, name="u0h")
                    u1h = pp.tile([P, LOC], F32, tag="agg", bufs=2, name="u1h")
                    uzh = pp.tile([2, LOC], F32, tag="aggz", bufs=1, name="uzh")
                    for m in range(NB):
                        ek = wk.tile([P, LOC], BF16, tag="wb512", bufs=6)
                        e_getter(ek, m)
                        last = m == NB - 1
                        nc.tensor.matmul(
                            u0h[:], h_aug[:, m, 0:P], ek[:],
                            start=(m == 0), stop=last,
                        )
                        nc.tensor.matmul(
                            u1h[:], h_aug[:, m, P : 2 * P], ek[:],
                            start=(m == 0), stop=last,
                        )
                        nc.tensor.matmul(
                            uzh[:], h_aug[:, m, HID : HID + 2], ek[:],
                            start=(m == 0), stop=last,
                        )
                    zrh = wk.tile([1, LOC], F32, tag="row", bufs=2)
                    nc.vector.reciprocal(zrh[:], uzh[0:1, :])
                    zbh = wk.tile([P, LOC], F32, tag="w512f", bufs=8)
                    nc.gpsimd.partition_broadcast(zbh[:], zrh[:])
                    for mt, um in enumerate((u0h, u1h)):
                        tn = wk.tile([P, LOC], F32R, tag="w512f", bufs=8)
                        nc.vector.tensor_mul(out=tn[:], in0=um[:], in1=zbh[:])
                        if first:
                            nc.vector.tensor_add(
                                out=outT[:, mt], in0=hT[:, mt], in1=tn[:]
                            )
                        else:
                            nc.vector.tensor_add(
                                out=outT[:, mt], in0=outT[:, mt], in1=tn[:]
                            )

                def make_e_get(mask_fp8):
                    def _get(ek, m):
                        mb = wk.tile([P, LOC], BF16, tag="wb512", bufs=6)
                        nc.vector.tensor_copy(mb[:], mask_fp8[:, m])
                        nc.vector.tensor_mul(out=ek[:], in0=expS[:, m], in1=mb[:])
                    return _get

                hop(make_e_get(M0), first=True)
                hop(make_e_get(M1), first=False)
                hop(make_e_get(M2), first=False)

            # =========== phase 7: output projection ===========
            py = pp.tile([P, LOC], F32, tag="bcast", bufs=1, name="py")
            for k in range(2):
                nc.tensor.matmul(
                    py[:],
                    Wo_sb[:, k, :],
                    outT[:, k, :],
                    start=(k == 0),
                    stop=(k == 1),
                )
            yt = sm.tile([P, LOC], F32, name="yt")
            nc.vector.tensor_scalar(
                yt[:], py[:], bo_sb[:, 0:1], None, mybir.AluOpType.add
            )
            nc.sync.dma_start(out_d[:, :], yt[:])

    nc.compile()
    return nc


def _get_nc():
    if "nc" not in _CACHE:
        _CACHE["nc"] = build_kernel()
    return _CACHE["nc"]


def kernel(X, A, W_s, r, W_l, W_out, b_out):
    global last_in_maps
    X = np.ascontiguousarray(X, dtype=np.float32)
    A = np.ascontiguousarray(A, dtype=np.float32)
    in_maps = []
    for c in range(NCORES):
        in_maps.append(
            {
                "X": X,
                "X_loc": np.ascontiguousarray(X[c * LOC : (c + 1) * LOC]),
                "A_blk": np.ascontiguousarray(A[c * LOC : (c + 1) * LOC]),
                "W_s": np.ascontiguousarray(W_s, dtype=np.float32),
                "r": np.ascontiguousarray(r, dtype=np.float32),
                "W_l": np.ascontiguousarray(W_l, dtype=np.float32),
                "W_out": np.ascontiguousarray(W_out, dtype=np.float32),
                "b_out": np.ascontiguousarray(b_out, dtype=np.float32),
            }
        )
    last_in_maps = in_maps
    nc = _get_nc()
    res = run_bass_kernel_spmd(nc, in_maps, core_ids=list(range(NCORES)))
    Y = np.empty((N, OUT_DIM), dtype=np.float32)
    for c in range(NCORES):
        Y[c * LOC : (c + 1) * LOC, :] = res.results[c]["out"].T
    return Y


if __name__ == "__main__":
    build_kernel()
    print("build OK")
